# revision 44
# baseline (speedup 1.0000x reference)
"""Multi-head attention (B=4, S=2048, D=1024, H=16, causal) on 8 TRN2 NeuronCores.

Sharding: core i handles batch i//2 and head-group i%2 (8 heads / 512 projection
columns). Each core computes a partial output projection over its 512 rows of Wo;
the host sums the two partials per batch and adds bo. No device collectives.

Per-core dataflow (bf16 matmuls, fp32 softmax):
  QT/KT = W-stationary projections of pre-transposed x; V in natural layout with
  an interleaved ones column per head (softmax denominator rides the AV matmul).
  Attention runs per HEAD-PAIR: the two heads' score matmuls (K=64 contraction)
  sit on PE row-groups 0-63 / 64-127 and are issued back-to-back, so the array
  computes both concurrently (row tiling) -- scores cost one head's time.
  Scores are [k, q] per k-tile in a shared [128, 1024] PSUM pair tile; one wide
  exp per k-tile; diagonal k-tiles trim the matmul/exp N-range to the unmasked
  q-columns and mask only the 128x128 triangle. AV accumulates [out^T | denom]
  per head (K=128, serial). j-major schedule: projections and yproj groups are
  woven between attend pairs as PE filler, so output stores spread across the
  whole kernel instead of a tail. Biases are compile-time dropped (the model
  fills them with zeros; nonzero bias falls back to a host path).
"""

import sys

for _p in ("/opt/trn_rl_repo",):
    if _p not in sys.path:
        sys.path.insert(0, _p)

import numpy as np
import ml_dtypes

BF16 = ml_dtypes.bfloat16

B, S, D = 4, 2048, 1024
H, HD = 16, 64
HPC = H // 2          # heads per core: 8
DPC = D // 2          # projection cols per core: 512
NCORES = 8
SCALE = 1.0 / np.sqrt(np.float32(HD))

_compiled = None


def _build():
    import concourse.bacc as bacc
    import concourse.mybir as mybir
    import concourse.tile as tile

    f32 = mybir.dt.float32
    bf = mybir.dt.bfloat16
    Exp = mybir.ActivationFunctionType.Exp
    Copy = mybir.ActivationFunctionType.Copy

    nc = bacc.Bacc("TRN2", target_bir_lowering=False, debug=False)

    xtq = nc.dram_tensor("xtq", [D, S], bf, kind="ExternalInput")
    xtk = nc.dram_tensor("xtk", [D, S], bf, kind="ExternalInput")
    xtv = nc.dram_tensor("xtv", [D, S], bf, kind="ExternalInput")
    wq = nc.dram_tensor("wq", [D, DPC], bf, kind="ExternalInput")
    wk = nc.dram_tensor("wk", [D, DPC], bf, kind="ExternalInput")
    wv = nc.dram_tensor("wv", [D, DPC], bf, kind="ExternalInput")
    wo = nc.dram_tensor("wo", [DPC, D], bf, kind="ExternalInput")
    dmask = nc.dram_tensor("dmask", [128, 256], bf, kind="ExternalInput")
    y = nc.dram_tensor("y", [S, D], bf, kind="ExternalOutput")

    NKD = D // 128        # 8 contraction tiles for projections
    NST = S // 128        # 16 seq tiles
    NSB = S // 512        # 4 seq blocks
    NHP = HPC // 2        # 4 head pairs

    with tile.TileContext(nc) as tc:
        with (
            tc.tile_pool(name="consts", bufs=1) as consts,
            tc.tile_pool(name="wqp", bufs=1) as wqp,
            tc.tile_pool(name="wkp", bufs=1) as wkp,
            tc.tile_pool(name="wvp", bufs=1) as wvp,
            tc.tile_pool(name="wop", bufs=1) as wop,
            tc.tile_pool(name="xt", bufs=1) as xtp,
            tc.tile_pool(name="qt", bufs=NHP) as qtp,
            tc.tile_pool(name="kt", bufs=NHP) as ktp,
            tc.tile_pool(name="vp", bufs=NST) as vpool,
            tc.tile_pool(name="ex", bufs=3) as expool,
            tc.tile_pool(name="ot", bufs=NHP) as otp,
            tc.tile_pool(name="ys", bufs=2) as ysp,
            tc.tile_pool(name="rc", bufs=2) as rcp,
            tc.tile_pool(name="rb", bufs=2) as rbp,
            tc.tile_pool(name="ps", bufs=2, space="PSUM") as psp,
            tc.tile_pool(name="sc", bufs=2, space="PSUM") as scp,
            tc.tile_pool(name="av", bufs=2, space="PSUM") as avp,
        ):
            # constants
            dmt = consts.tile([128, 256], bf, tag="dmt")
            nc.sync.dma_start(dmt[:], dmask.ap()[:])
            # PE warmup: junk matmuls while input DMAs land, so HAM ramps
            # toward full clock before the first real projection group
            warm = consts.tile([128, 512], bf, tag="warm")
            nc.gpsimd.memset(warm[:], 0.25)
            wps = psp.tile([128, 512], f32, name="wps", tag="ps")
            for _ in range(24):
                nc.tensor.matmul(wps[:], warm[:, 0:128], warm[:], start=True, stop=True)

            # ---- DMAs, in consumption order.
            # wv + xtv sb0 first (V groups are the first real PE work), then
            # wq/wk + xtq sb0 (qt groups), xtk sb0 (kt groups; shares SBUF
            # slots with xtv so the DMA waits only on V-group readers), wo,
            # then the sb1..3 quarters j-major.
            # weights batch into one ~1MB DMA each (same [p, kd, c] pattern)
            def make_w(src_t, pool, nm, nrow, ncol):
                w = pool.tile([128, nrow * ncol], bf, name=nm, tag=nm)
                dst = w[:].rearrange("p (kd c) -> p kd c", kd=nrow)
                src = src_t.ap().rearrange("(kd p) c -> p kd c", kd=nrow)
                nc.sync.dma_start(dst, src)
                return w

            wv_t = make_w(wv, wvp, "wv", NKD, DPC)

            # x inputs load one seq-block at a time as a single 1MB DMA with a
            # [128p, 8kd, 512c] access pattern (per-DMA cost is ~2us fixed +
            # bytes/436GB/s, so 1MB transfers run near peak while arriving in
            # the same need-order as per-(kd,sb) quarters would)
            def make_sb(src_t, prefix, sb):
                xt = xtp.tile([128, NKD * 512], bf, name=f"{prefix}{sb}",
                              tag=f"{prefix}{sb}", bufs=1)
                dst = xt[:].rearrange("p (kd c) -> p kd c", kd=NKD)
                src = src_t.ap().rearrange("(kd p) s -> p kd s", kd=NKD)
                nc.sync.dma_start(dst, src[:, :, sb * 512:(sb + 1) * 512])
                return xt

            def x_slice(xsb, kd, c0, c1):
                return xsb[:, kd * 512 + c0:kd * 512 + c1]

            xtv_b = [None] * NSB
            xtq_b = [None] * NSB
            xtk_b = [None] * NSB
            xtv_b[0] = make_sb(xtv, "xv", 0)
            wq_t = make_w(wq, wqp, "wq", NKD, DPC)
            wk_t = make_w(wk, wkp, "wk", NKD, DPC)
            xtq_b[0] = make_sb(xtq, "xq", 0)
            xtk_b[0] = make_sb(xtk, "xk", 0)
            wo_t = make_w(wo, wop, "wo", 4, D)
            xtv_b[1] = make_sb(xtv, "xv", 1)
            xtq_b[1] = make_sb(xtq, "xq", 1)
            xtk_b[1] = make_sb(xtk, "xk", 1)
            xtv_b[2] = make_sb(xtv, "xv", 2)
            xtv_b[3] = make_sb(xtv, "xv", 3)
            xtq_b[2] = make_sb(xtq, "xq", 2)
            xtk_b[2] = make_sb(xtk, "xk", 2)
            xtq_b[3] = make_sb(xtq, "xq", 3)

            # ---- V projection groups (natural layout, [8 heads x 65] + ones)
            vts = [vpool.tile([128, HPC * 65], bf, name=f"v{st}", tag="v")
                   for st in range(NST)]

            def v_group(st):
                def group():
                    ps = psp.tile([128, 512], f32, name="psv", tag="ps")
                    for kd in range(NKD):
                        nc.tensor.matmul(
                            ps[:],
                            x_slice(xtv_b[st // 4], kd,
                                    (st % 4) * 128, (st % 4 + 1) * 128),
                            wv_t[:, kd * DPC:(kd + 1) * DPC],
                            start=(kd == 0), stop=(kd == NKD - 1),
                        )
                    vt = vts[st]
                    v3 = vt[:].rearrange("p (h c) -> p h c", h=HPC, c=65)
                    nc.vector.tensor_copy(
                        v3[:, :, 0:64],
                        ps[:].rearrange("p (h c) -> p h c", h=HPC, c=64),
                    )
                    nc.gpsimd.memset(v3[:, :, 64:65], 1.0)
                return group

            # ---- QT / KT projection groups
            qts, kts = [], []
            for pool, lst, nm in ((qtp, qts, "qt"), (ktp, kts, "kt")):
                for hp in range(NHP):
                    lst.append(pool.tile([128, S], bf, name=f"{nm}{hp}", tag=nm))

            def proj_group(xb, wts, dest, hp, sb):
                def group():
                    ps = psp.tile([128, 512], f32, name="psq", tag="ps")
                    for kd in range(NKD):
                        nc.tensor.matmul(
                            ps[:],
                            wts[:, kd * DPC + hp * 128:kd * DPC + (hp + 1) * 128],
                            x_slice(xb[sb], kd, 0, 512),
                            start=(kd == 0), stop=(kd == NKD - 1),
                        )
                    nc.vector.tensor_copy(dest[:, sb * 512:(sb + 1) * 512], ps[:])
                return group

            ots = [otp.tile([128, S], bf, name=f"ot{i}", tag="ot") for i in range(NHP)]

            def yproj_group(st, eb):
                def group():
                    ps = psp.tile([128, 512], f32, name="psy", tag="ps")
                    for hp in range(NHP):
                        nc.tensor.matmul(
                            ps[:],
                            ots[hp][:, st * 128:(st + 1) * 128],
                            wo_t[:, hp * D + eb * 512:hp * D + (eb + 1) * 512],
                            start=(hp == 0), stop=(hp == NHP - 1),
                        )
                    ys = ysp.tile([128, 512], bf, name="ys", tag="ys")
                    # evict on alternating engines; the store DMA is issued
                    # from the evicting engine so it never queues behind
                    # gated input DMAs on the SP engine
                    if (st + eb) % 2 == 0:
                        nc.vector.tensor_copy(ys[:], ps[:])
                    else:
                        nc.scalar.activation(ys[:], ps[:], Copy)
                    nc.sync.dma_start(
                        y.ap()[st * 128:(st + 1) * 128, eb * 512:(eb + 1) * 512],
                        ys[:],
                    )
                return group

            # upfront: V st0-3 + qt0/kt0 sb0; everything else is filler
            for st in range(4):
                v_group(st)()
            proj_group(xtq_b, wq_t, qts[0], 0, 0)()
            proj_group(xtk_b, wk_t, kts[0], 0, 0)()

            # xtk sb3 reuses xtv sb0's slot: its V0-3 readers are emitted
            # above, so the DMA's write-after-read wait is tracked and long
            # satisfied by the time this transfer is reached (it is the last
            # input needed, for the kt sb3 projections late in the schedule)
            xtk_b[3] = xtp.tile([128, NKD * 512], bf, name="xk3",
                                tag="xv0", bufs=1)
            nc.sync.dma_start(
                xtk_b[3][:].rearrange("p (kd c) -> p kd c", kd=NKD),
                xtk.ap().rearrange("(kd p) s -> p kd s", kd=NKD)[
                    :, :, 3 * 512:4 * 512])

            # filler, j-major: per j, projections for the NEXT j's attends,
            # with the remaining V groups woven through the early phases
            # (min_pair, group): a paced pop skips groups whose input DMA
            # cannot have landed yet, so they never block the PE queue
            gate = {1: 3, 2: 5, 3: 8}
            filler = []
            filler += [(0, proj_group(xtq_b, wq_t, qts[1], 1, 0)),
                       (0, proj_group(xtk_b, wk_t, kts[1], 1, 0)),
                       (2, v_group(4)), (2, v_group(5)),
                       (0, proj_group(xtq_b, wq_t, qts[2], 2, 0)),
                       (0, proj_group(xtk_b, wk_t, kts[2], 2, 0)),
                       (2, v_group(6)), (2, v_group(7)),
                       (0, proj_group(xtq_b, wq_t, qts[3], 3, 0)),
                       (0, proj_group(xtk_b, wk_t, kts[3], 3, 0))]
            for sb in range(1, NSB):
                g = gate[sb]
                for hp in range(NHP):
                    filler.append((g, proj_group(xtq_b, wq_t, qts[hp], hp, sb)))
                    filler.append((g, proj_group(xtk_b, wk_t, kts[hp], hp, sb)))
                    if hp == 1 and sb < 3:
                        filler.append((gate[sb + 1], v_group(4 * sb + 4)))
                        filler.append((gate[sb + 1], v_group(4 * sb + 5)))
                    if hp == 2 and sb < 3:
                        filler.append((gate[sb + 1], v_group(4 * sb + 6)))
                        filler.append((gate[sb + 1], v_group(4 * sb + 7)))

            # need[(hp, j)]: filler index that must be emitted before
            # pair(hp, j); computed from the construction above
            need = {
                (0, 0): 0, (1, 0): 2, (2, 0): 6, (3, 0): 10,
                (0, 1): 12, (1, 1): 14, (2, 1): 18, (3, 1): 22,
                (0, 2): 24, (1, 2): 26, (2, 2): 30, (3, 2): 34,
                (0, 3): 36, (1, 3): 38, (2, 3): 40, (3, 3): 42,
            }
            assert len(filler) == 42

            emitted = [0]

            def pop_filler_until(n):
                while emitted[0] < min(n, len(filler)):
                    filler[emitted[0]][1]()
                    emitted[0] += 1

            # yproj groups become eligible filler two pairs after the pair
            # that completed their seq block, so the normalize chains have
            # executed (not merely been emitted) by the time PE reaches them
            pair_no = [0]
            yfill = []   # (eligible_pair_no, group)
            ktile_no = [0]

            def maybe_filler(force=False):
                if not force and ktile_no[0] % 2 != 0:
                    return
                if (emitted[0] < len(filler)
                        and filler[emitted[0]][0] <= pair_no[0]):
                    pop_filler_until(emitted[0] + 1)
                elif (yfill and yfill[0][0] <= pair_no[0]
                      and len(yfill) > 8):
                    # keep >=8 yproj groups in reserve: they run during the
                    # last pair's normalize chains so PE never drains dry
                    yfill.pop(0)[1]()

            dmt3 = dmt[:].rearrange("p (g q) -> p g q", g=2)

            def attend_pair(hp, j, before_norm=None):
                pop_filler_until(need[(hp, j)])
                qt_p = qts[hp]
                kt_p = kts[hp]
                hA, hB = 2 * hp, 2 * hp + 1
                avA = avp.tile([128, 512], f32, name="avA", tag="av")
                avB = avp.tile([128, 512], f32, name="avB", tag="av")
                nkt = 4 * (j + 1)
                # diagonal k-tiles first: their trimmed/masked exp overlaps
                # later k-tiles; first and last entries are full-width so the
                # PSUM accumulation group starts/stops on the whole av region
                kt_order = list(range(4 * j, nkt)) + list(range(0, 4 * j))
                prev_av = None

                def make_av(ex, kti, q0, first, last):
                    def emit():
                        ex2 = ex[:].rearrange("p (g q) -> p g q", g=2)
                        nc.tensor.matmul(
                            avA[0:65, q0:512],
                            vts[kti][:, hA * 65:(hA + 1) * 65],
                            ex2[:, 0, q0:512],
                            start=first, stop=last,
                        )
                        nc.tensor.matmul(
                            avB[0:65, q0:512],
                            vts[kti][:, hB * 65:(hB + 1) * 65],
                            ex2[:, 1, q0:512],
                            start=first, stop=last,
                        )
                    return emit

                first = True
                for t, kti in enumerate(kt_order):
                    rr = kti - 4 * j          # >=0: diagonal k-tile
                    q0 = rr * 128 if rr > 0 else 0
                    sc = scp.tile([128, 1024], f32, name="sc", tag="sc")
                    # the two heads' score matmuls contract K=64 on PE row
                    # groups 0-63 / 64-127 and are issued adjacently, so the
                    # array runs them concurrently (row tiling)
                    nc.tensor.matmul(
                        sc[:, q0:512],
                        kt_p[0:64, kti * 128:(kti + 1) * 128],
                        qt_p[0:64, j * 512 + q0:(j + 1) * 512],
                        start=True, stop=True,
                    )
                    nc.tensor.matmul(
                        sc[:, 512 + q0:1024],
                        kt_p[64:128, kti * 128:(kti + 1) * 128],
                        qt_p[64:128, j * 512 + q0:(j + 1) * 512],
                        start=True, stop=True,
                    )
                    ex = expool.tile([128, 1024], bf, name="ex")
                    if q0 == 0:
                        nc.scalar.activation(ex[:], sc[:], Exp, scale=float(SCALE))
                    else:
                        nc.scalar.activation(ex[:, q0:512], sc[:, q0:512],
                                             Exp, scale=float(SCALE))
                        nc.scalar.activation(ex[:, 512 + q0:1024],
                                             sc[:, 512 + q0:1024],
                                             Exp, scale=float(SCALE))
                    if rr >= 0:   # causal mask: both heads' 128x128 triangles
                        ex3 = ex[:].rearrange("p (g q) -> p g q", g=2)
                        nc.vector.tensor_mul(
                            ex3[:, :, rr * 128:(rr + 1) * 128],
                            ex3[:, :, rr * 128:(rr + 1) * 128],
                            dmt3[:, :, :],
                        )
                    ktile_no[0] += 1
                    maybe_filler()
                    if prev_av is not None:
                        prev_av()
                    prev_av = make_av(ex, kti, q0, first, t == nkt - 1)
                    first = False
                prev_av()
                if before_norm is not None:
                    before_norm()

                # normalize both heads: evict av, reciprocal of the denom row
                # in place, partition broadcast, multiply into the ot tile
                # normalize both heads: evict av (row 64 = denominator),
                # reciprocal via a [1,512]->[128,4] reshape DMA issued on the
                # otherwise-idle SP engine, broadcast the reciprocal row into
                # partitions 64-127 of the av PSUM tile with a K=1 matmul,
                # then one DVE multiply writes the ot slice. No gpsimd: its
                # queue was the serialized train that stalled PE via the av
                # slot rotation.
                # normalize both heads: evict av (row 64 = denominator),
                # reciprocal via a [1,512]->[128,4] reshape DMA (issued on the
                # otherwise-idle SP engine, NOT gpsimd -- its descriptor-gen
                # cost made the per-pair chains outrun the attends and stall
                # PE through the av slot rotation), gpsimd partition
                # broadcast, DVE multiply into the ot tile. The two heads'
                # chains are interleaved step-by-step so their hops pipeline.
                chain = []
                for h, av in ((hA, avA), (hB, avB)):
                    avs = ysp.tile([65, 512], bf, name="avs", tag="avs", bufs=4)
                    rsh = rcp.tile([128, 4], bf, name="rsh", tag="rsh", bufs=4)
                    rr_t = rcp.tile([128, 4], bf, name="rr", tag="rr", bufs=4)
                    rrow = rcp.tile([1, 512], bf, name="rrow", tag="rrow", bufs=4)
                    rb = rbp.tile([64, 512], bf, name="rb", tag="rb", bufs=4)
                    chain.append((h, av, avs, rsh, rr_t, rrow, rb))
                with nc.allow_low_precision(
                        reason="bf16 softmax normalization; rel-err budget 2e-2"):
                    for h, av, avs, rsh, rr_t, rrow, rb in chain:
                        nc.vector.tensor_copy(avs[:], av[0:65, :])
                        nc.gpsimd.dma_start(rsh[:], avs[64:65, :])
                    for h, av, avs, rsh, rr_t, rrow, rb in chain:
                        nc.vector.reciprocal(rr_t[:], rsh[:])
                    for h, av, avs, rsh, rr_t, rrow, rb in chain:
                        nc.gpsimd.dma_start(rrow[:], rr_t[:])
                    for h, av, avs, rsh, rr_t, rrow, rb in chain:
                        nc.gpsimd.partition_broadcast(rb[:], rrow[:], channels=64)
                    for h, av, avs, rsh, rr_t, rrow, rb in chain:
                        base = (h % 2) * 64
                        nc.vector.tensor_mul(
                            ots[hp][base:base + 64, j * 512:(j + 1) * 512],
                            avs[0:64, :],
                            rb[:],
                        )
                pair_no[0] += 1

            # yproj eligibility: j0 blocks run early in j2 (late enough that
            # their normalize chains have long executed); j1/j2 blocks are
            # held for the j3 phase (projection filler is exhausted by then)
            elig = {0: 6, 1: 12, 2: 13, 3: 99}

            def drain_reserve():
                # emit the reserved yproj groups BEFORE the final normalize
                # chains: dependency tracking is tile-granular on ots, so a
                # yproj emitted after a later ot write would wait on it
                while yfill:
                    yfill.pop(0)[1]()

            for j in range(NSB):
                for hp in range(NHP):
                    attend_pair(hp, j,
                                before_norm=drain_reserve
                                if (hp, j) == (NHP - 1, NSB - 1) else None)
                for st in range(4 * j, 4 * j + 4):
                    for eb in range(2):
                        yfill.append((elig[j], yproj_group(st, eb)))
            pop_filler_until(len(filler))
            for _, g in yfill:
                g()

    nc.compile()
    return nc


def _diag_mask():
    tri = np.triu(np.ones((128, 128), np.float32))  # mask[k,q]=1 iff k<=q
    return np.concatenate([tri, tri], axis=1).astype(BF16)


def _shard_inputs(q_in, k_in, v_in, Wq, Wk, Wv, Wo):
    dm = _diag_mask()
    in_maps = []
    for core in range(NCORES):
        b, g = core // 2, core % 2
        cs = slice(g * DPC, (g + 1) * DPC)
        in_maps.append({
            "xtq": np.ascontiguousarray(q_in[b].T).astype(BF16),
            "xtk": np.ascontiguousarray(k_in[b].T).astype(BF16),
            "xtv": np.ascontiguousarray(v_in[b].T).astype(BF16),
            "wq": Wq[:, cs].astype(BF16),
            "wk": Wk[:, cs].astype(BF16),
            "wv": Wv[:, cs].astype(BF16),
            "wo": np.ascontiguousarray(Wo[cs, :]).astype(BF16),
            "dmask": dm,
        })
    return in_maps


def _host_fallback(q_in, k_in, v_in, Wq, bq, Wk, bk, Wv, bv, Wo, bo):
    # only reached if a caller passes nonzero projection biases (the model
    # under test always has zero biases); plain numpy reference
    out = np.empty((B, S, D), np.float32)
    causal = np.arange(S)[None, :] <= np.arange(S)[:, None]
    for b in range(B):
        Q = (q_in[b] @ Wq + bq).reshape(S, H, HD).transpose(1, 0, 2)
        K = (k_in[b] @ Wk + bk).reshape(S, H, HD).transpose(1, 0, 2)
        V = (v_in[b] @ Wv + bv).reshape(S, H, HD).transpose(1, 0, 2)
        oh = np.empty((H, S, HD), np.float32)
        for h in range(H):
            sc = (Q[h] @ K[h].T) * SCALE
            e = np.where(causal, np.exp(sc - sc.max(1, keepdims=True)), 0.0)
            oh[h] = (e / e.sum(1, keepdims=True)) @ V[h]
        out[b] = oh.transpose(1, 0, 2).reshape(S, D) @ Wo + bo
    return out


def kernel(q_in, k_in, v_in, Wq, bq, Wk, bk, Wv, bv, Wo, bo, _trace=False):
    from concourse.bass_utils import run_bass_kernel_spmd

    args = [np.asarray(a, np.float32) for a in
            (q_in, k_in, v_in, Wq, bq, Wk, bk, Wv, bv, Wo, bo)]
    if any(np.abs(a).max() > 0 for a in (args[4], args[6], args[8])):
        return _host_fallback(*args)

    global _compiled
    if _compiled is None:
        _compiled = _build()

    in_maps = _shard_inputs(args[0], args[1], args[2], args[3], args[5],
                            args[7], args[9])
    res = run_bass_kernel_spmd(
        _compiled, in_maps, core_ids=list(range(NCORES)), trace=_trace,
    )
    bo_f = args[10]
    out = np.empty((B, S, D), np.float32)
    for b in range(B):
        out[b] = (res.results[2 * b]["y"].astype(np.float32)
                  + res.results[2 * b + 1]["y"].astype(np.float32) + bo_f)
    if _trace:
        kernel.last_results = res
    return out
